# revision 19
# baseline (speedup 1.0000x reference)
"""Grouped GRU cell (nn_GRUCell) on 8 Trainium2 NeuronCores.

Problem shape: B=1024, I=256 groups, D=128.
  r   = sigmoid(X[:,i,None]*W_r[i] + hg @ U_r[i] + b_r[i])
  z   = sigmoid(X[:,i,None]*W_z[i] + hg @ U_z[i] + b_z[i])
  h~  = tanh   (X[:,i,None]*W_h[i] + (r*hg) @ U_h[i] + b_h[i])
  h'  = z*hg + (1-z)*h~
Outputs: (h', h~), both [B, I*D].

Sharding: groups are fully independent -> 32 groups per core, no collectives.

Device layout: tiles are [D(part)=128, B(free)=1024] per group; host
pre-permutes h to [group, D, B] and permutes outputs back.  All DMA
payloads are fp16 (tolerance 2e-2, we land ~2e-3), halving HBM traffic
and enabling the DVE 2x packed mode for the elementwise ops.

PE clock (measured on this pod): the HAM activity monitor only
un-throttles 1.2->2.4GHz after ~3.4us of high PE-array activity, where
activity scales with K (rows driven) and is data-independent; K<=2
matmuls and idle bubbles keep it cold forever.  So:
  - every matmul is K=128: the X*W rank-1 term uses a host-built lhsT
    with W in row 0 and 127 zero rows ("wpad"), and a device-resident
    moving tile with X_g DMA'd into row 0 over 127 memset-zero rows
    ("xmov", double-buffered); zero rows contribute nothing.
  - b rides the activation's per-partition bias operand (free).
  - a warmup burst of K=128 zero matmuls trips the HAM at t=0, and
    1-2 filler matmuls (K=128 zeros -> scratch PSUM bank) sit at each
    natural pipeline stall so the activity window never drops.
Warm, the PE issues one N=512 matmul per ~259ns with LDWEIGHTS hidden.

Steady state is ScalarE-bound at ~3.6-4us/group: sigmoid(pr),
sigmoid(pz), tanh on the two 512-wide halves of ph (half-bank ph tiles
free up the scratch bank: pr2 + pz2 + ph 3x1 + scratch1 = 8 banks).
DVE: rh=r*h, d=h-h~, t=z*d, hn=t+h~ as plain fp16 tensor_tensor (2x
mode).  Pool: output DMA triggers only.
"""

import os
from contextlib import ExitStack

import numpy as np

import concourse.bass as bass
import concourse.tile as tile
from concourse import bacc, mybir
from concourse.bass_utils import run_bass_kernel_spmd

B = 1024
I = 256
D = 128
NCORES = 8
GPC = I // NCORES  # 32 groups per core
NCHUNK = 2
CHUNK = B // NCHUNK  # 512

F16 = mybir.dt.float16
F32 = mybir.dt.float32

N_WARMUP = 12
FILL_PR = 1
FILL_PZ = 1
FILL_PH = 1

_PROGRAM = None


def _build_program():
    nc = bacc.Bacc(
        "TRN2",
        target_bir_lowering=False,
        debug=False,
        enable_asserts=False,
    )

    hT_d = nc.dram_tensor("hT", [GPC, D, B], F16, kind="ExternalInput").ap()
    u_d = nc.dram_tensor("ucat", [D, GPC * 3 * D], F16, kind="ExternalInput").ap()
    # W in row 0, 127 zero rows below (K=128-padded rank-1 stationary).
    wpad_d = nc.dram_tensor("wpad", [D, GPC * 3 * D], F16, kind="ExternalInput").ap()
    xrow_d = nc.dram_tensor("xrow", [GPC, B], F16, kind="ExternalInput").ap()
    # Full [D,B] images (row0=X_g, rest zeros) for the first XBUF groups --
    # loading these via DMA removes the xmov memset from the startup path.
    xfull_d = nc.dram_tensor("xfull", [8, D, B], F16, kind="ExternalInput").ap()
    b_d = nc.dram_tensor("bcat", [D, GPC * 3], F32, kind="ExternalInput").ap()
    hnT_d = nc.dram_tensor("hnT", [GPC, D, B], F16, kind="ExternalOutput").ap()
    htT_d = nc.dram_tensor("htT", [GPC, D, B], F16, kind="ExternalOutput").ap()

    sig = mybir.ActivationFunctionType.Sigmoid
    tanh = mybir.ActivationFunctionType.Tanh

    with tile.TileContext(nc) as tc, ExitStack() as ctx:
        u_pool = ctx.enter_context(tc.tile_pool(name="u", bufs=1))
        wpad_pool = ctx.enter_context(tc.tile_pool(name="wp", bufs=1))
        xmov_pool = ctx.enter_context(tc.tile_pool(name="xm", bufs=1))
        b_pool = ctx.enter_context(tc.tile_pool(name="b", bufs=1))
        warm_pool = ctx.enter_context(tc.tile_pool(name="warm", bufs=1))
        hT_pool = ctx.enter_context(tc.tile_pool(name="hT", bufs=6))
        ps_pool = ctx.enter_context(tc.tile_pool(name="ps", bufs=3, space="PSUM"))
        fill_pool = ctx.enter_context(tc.tile_pool(name="fil", bufs=1, space="PSUM"))
        act_pool = ctx.enter_context(tc.tile_pool(name="act", bufs=4))
        mid_pool = ctx.enter_context(tc.tile_pool(name="mid", bufs=6))
        out_pool = ctx.enter_context(tc.tile_pool(name="out", bufs=4))

        # K=128 zero operands for warmup/filler matmuls.
        zf = warm_pool.tile([D, D + CHUNK], F16)
        nc.vector.memset(zf[:], 0)
        fill_ps = fill_pool.tile([D, CHUNK], F32)

        def filler(n):
            for _ in range(n):
                nc.tensor.matmul(
                    fill_ps[:], lhsT=zf[:, :D], rhs=zf[:, D:],
                    start=True, stop=True, skip_group_check=True,
                )

        filler(N_WARMUP)

        # xmov: 8-buffered [D, B] moving operand for the rank-1 term;
        # rows 1..127 stay zero forever, row 0 gets X_g per group.
        # XBUF must exceed PF+3 so the prefetch for g+PF never lands in a
        # buffer still readable by stageB(g-1).
        XBUF = 8
        xmov = xmov_pool.tile([D, XBUF * B], F16)

        b_sb = b_pool.tile([D, GPC * 3], F32)
        nc.sync.dma_start(b_sb[:], b_d[:])
        actwarm = warm_pool.tile([D, 1], F16)
        nc.scalar.activation(actwarm[:], b_sb[:, 0:1], sig)
        u_sb = u_pool.tile([D, GPC * 3 * D], F16)
        wpad_sb = wpad_pool.tile([D, GPC * 3 * D], F16)
        NCHK = 4
        UCH = GPC // NCHK * 3 * D

        def load_chunk(c):
            # u/wpad for one 4-group slice, issued lazily so the first hT
            # DMA isn't queued behind 6 MiB of weights.
            nc.sync.dma_start(u_sb[:, c * UCH : (c + 1) * UCH], u_d[:, c * UCH : (c + 1) * UCH])
            nc.sync.dma_start(
                wpad_sb[:, c * UCH : (c + 1) * UCH], wpad_d[:, c * UCH : (c + 1) * UCH]
            )

        def prefetch(g):
            hT = hT_pool.tile([D, B], F16, tag="hT", name=f"hT{g}")
            nc.sync.dma_start(hT[:], hT_d[g])
            xoff = (g % XBUF) * B
            if g < XBUF:
                nc.sync.dma_start(xmov[:, xoff : xoff + B], xfull_d[g])
            else:
                nc.sync.dma_start(xmov[0:1, xoff : xoff + B], xrow_d[g : g + 1, :])
            return hT

        def gemm(psum, g, gate, moving):
            u_g = u_sb[:, (g * 3 + gate) * D : (g * 3 + gate + 1) * D]
            w_g = wpad_sb[:, (g * 3 + gate) * D : (g * 3 + gate + 1) * D]
            xoff = (g % XBUF) * B
            for c in range(NCHUNK):
                sl = slice(c * CHUNK, (c + 1) * CHUNK)
                nc.tensor.matmul(
                    psum[:, sl], lhsT=u_g, rhs=moving[:, sl],
                    start=True, stop=False,
                )
            for c in range(NCHUNK):
                sl = slice(c * CHUNK, (c + 1) * CHUNK)
                nc.tensor.matmul(
                    psum[:, sl], lhsT=w_g,
                    rhs=xmov[:, xoff + c * CHUNK : xoff + (c + 1) * CHUNK],
                    start=False, stop=True,
                )

        def bias(g, gate):
            return b_sb[:, g * 3 + gate : g * 3 + gate + 1]

        def stageA(g, hT):
            filler(FILL_PR)
            pr = ps_pool.tile([D, B], F32, tag="ps", name=f"pr{g}")
            gemm(pr, g, 0, hT)
            rt = act_pool.tile([D, B], F16, tag="rt", name=f"rt{g}")
            nc.scalar.activation(rt[:], pr[:], sig, bias=bias(g, 0))
            rh = mid_pool.tile([D, B], F16, tag="rh", name=f"rh{g}")
            nc.vector.tensor_mul(rh[:], rt[:], hT[:])

            filler(FILL_PZ)
            pz = ps_pool.tile([D, B], F32, tag="ps", name=f"pz{g}")
            gemm(pz, g, 1, hT)
            zt = act_pool.tile([D, B], F16, tag="zt", name=f"zt{g}")
            nc.scalar.activation(zt[:], pz[:], sig, bias=bias(g, 1))
            return dict(g=g, hT=hT, zt=zt, rh=rh)

        def stageB(s, last=False):
            g = s["g"]
            filler(FILL_PH)
            ht = out_pool.tile([D, B], F16, tag="ht", name=f"ht{g}")
            ph = ps_pool.tile([D, B], F32, tag="ps", name=f"ph{g}")
            gemm(ph, g, 2, s["rh"])
            nc.scalar.activation(ht[:], ph[:], tanh, bias=bias(g, 2))
            nc.gpsimd.dma_start(htT_d[g], ht[:])
            d = mid_pool.tile([D, B], F16, tag="d", name=f"d{g}")
            t = mid_pool.tile([D, B], F16, tag="t", name=f"t{g}")
            hn = out_pool.tile([D, B], F16, tag="hn", name=f"hn{g}")
            # For the last group, process the blend in halves so the final
            # output DMA starts ~1.5us earlier.
            slices = (
                [slice(0, CHUNK), slice(CHUNK, B)] if last else [slice(0, B)]
            )
            for sl in slices:
                nc.vector.tensor_sub(d[:, sl], s["hT"][:, sl], ht[:, sl])
                nc.vector.tensor_mul(t[:, sl], s["zt"][:, sl], d[:, sl])
                nc.vector.tensor_add(hn[:, sl], t[:, sl], ht[:, sl])
                if last:
                    nc.sync.dma_start(hnT_d[g][:, sl], hn[:, sl])
            if not last:
                nc.gpsimd.dma_start(hnT_d[g], hn[:])

        PF = 4  # DMA prefetch distance in groups
        load_chunk(0)
        hTs = {g: prefetch(g) for g in range(PF)}
        pend = None
        for g in range(GPC):
            if g + PF < GPC:
                hTs[g + PF] = prefetch(g + PF)
            if g % 8 == 5 and g // 8 + 1 < NCHK:
                load_chunk(g // 8 + 1)
            cur = stageA(g, hTs.pop(g))
            if pend is not None:
                stageB(pend)
            pend = cur
        stageB(pend, last=True)

    nc.compile()
    return nc


def _get_program():
    global _PROGRAM
    if _PROGRAM is None:
        _PROGRAM = _build_program()
    return _PROGRAM


LAST_EXEC_TIME_NS = None
LAST_RESULTS = None


def kernel(X, h, W_r, W_z, W_h, U_r, U_z, U_h, b_r, b_z, b_h):
    global LAST_EXEC_TIME_NS, LAST_RESULTS
    X = np.asarray(X, dtype=np.float32)
    h = np.asarray(h, dtype=np.float32)
    W = np.stack([np.asarray(W_r), np.asarray(W_z), np.asarray(W_h)], axis=1).astype(
        np.float32
    )  # [I, 3, 1, D]
    U = np.stack([np.asarray(U_r), np.asarray(U_z), np.asarray(U_h)], axis=1).astype(
        np.float32
    )  # [I, 3, D, D]
    bb = np.stack([np.asarray(b_r), np.asarray(b_z), np.asarray(b_h)], axis=1).astype(
        np.float32
    )  # [I, 3, D]

    hT = np.ascontiguousarray(h.reshape(B, I, D).transpose(1, 2, 0)).astype(np.float16)
    XT = np.ascontiguousarray(X.T).astype(np.float16)  # [I, B]

    in_maps = []
    for c in range(NCORES):
        sl = slice(c * GPC, (c + 1) * GPC)
        u_core = U[sl]  # [GPC, 3, D(k), D(d)]
        u_sb = np.ascontiguousarray(
            u_core.transpose(2, 0, 1, 3).reshape(D, GPC * 3 * D)
        ).astype(np.float16)
        wpad = np.zeros((D, GPC * 3 * D), dtype=np.float16)
        wpad[0, :] = W[sl, :, 0, :].reshape(GPC * 3 * D)
        bcat = np.ascontiguousarray(
            bb[sl].reshape(GPC * 3, D).T
        ).astype(np.float32)  # [D, GPC*3]
        xrow = np.ascontiguousarray(XT[sl])
        xfull = np.zeros((8, D, B), dtype=np.float16)
        xfull[:, 0, :] = xrow[:8]
        in_maps.append(
            {
                "hT": np.ascontiguousarray(hT[sl]),
                "ucat": u_sb,
                "wpad": wpad,
                "xrow": xrow,
                "bcat": bcat,
                "xfull": xfull,
            }
        )

    nc = _get_program()
    trace = bool(int(os.environ.get("KERNEL_TRACE", "0")))
    res = run_bass_kernel_spmd(nc, in_maps, core_ids=list(range(NCORES)), trace=trace)
    LAST_EXEC_TIME_NS = res.exec_time_ns
    LAST_RESULTS = res

    hnT = np.concatenate([res.results[c]["hnT"] for c in range(NCORES)], axis=0)
    htT = np.concatenate([res.results[c]["htT"] for c in range(NCORES)], axis=0)
    h_new = (
        np.ascontiguousarray(hnT.transpose(2, 0, 1)).reshape(B, I * D).astype(np.float32)
    )
    h_tilde = (
        np.ascontiguousarray(htT.transpose(2, 0, 1)).reshape(B, I * D).astype(np.float32)
    )
    return h_new, h_tilde


# revision 20
# speedup vs baseline: 1.0215x; 1.0215x over previous
"""Grouped GRU cell (nn_GRUCell) on 8 Trainium2 NeuronCores.

Problem shape: B=1024, I=256 groups, D=128.
  r   = sigmoid(X[:,i,None]*W_r[i] + hg @ U_r[i] + b_r[i])
  z   = sigmoid(X[:,i,None]*W_z[i] + hg @ U_z[i] + b_z[i])
  h~  = tanh   (X[:,i,None]*W_h[i] + (r*hg) @ U_h[i] + b_h[i])
  h'  = z*hg + (1-z)*h~
Outputs: (h', h~), both [B, I*D].

Sharding: groups are fully independent -> 32 groups per core, no collectives.

Device layout: tiles are [D(part)=128, B(free)=1024] per group; host
pre-permutes h to [group, D, B] and permutes outputs back.  All DMA
payloads are fp16 (tolerance 2e-2, we land ~2e-3), halving HBM traffic
and enabling the DVE 2x packed mode for the elementwise ops.

PE clock (measured on this pod): the HAM activity monitor only
un-throttles 1.2->2.4GHz after ~3.4us of high PE-array activity, where
activity scales with K (rows driven) and is data-independent; K<=2
matmuls and idle bubbles keep it cold forever.  So:
  - every matmul is K=128: the X*W rank-1 term uses a host-built lhsT
    with W in row 0 and 127 zero rows ("wpad"), and a device-resident
    moving tile with X_g DMA'd into row 0 over 127 memset-zero rows
    ("xmov", double-buffered); zero rows contribute nothing.
  - b rides the activation's per-partition bias operand (free).
  - a warmup burst of K=128 zero matmuls trips the HAM at t=0, and
    1-2 filler matmuls (K=128 zeros -> scratch PSUM bank) sit at each
    natural pipeline stall so the activity window never drops.
Warm, the PE issues one N=512 matmul per ~259ns with LDWEIGHTS hidden.

Steady state is ScalarE-bound at ~3.6-4us/group: sigmoid(pr),
sigmoid(pz), tanh on the two 512-wide halves of ph (half-bank ph tiles
free up the scratch bank: pr2 + pz2 + ph 3x1 + scratch1 = 8 banks).
DVE: rh=r*h, d=h-h~, t=z*d, hn=t+h~ as plain fp16 tensor_tensor (2x
mode).  Pool: output DMA triggers only.
"""

import os
from contextlib import ExitStack

import numpy as np

import concourse.bass as bass
import concourse.tile as tile
from concourse import bacc, mybir
from concourse.bass_utils import run_bass_kernel_spmd

B = 1024
I = 256
D = 128
NCORES = 8
GPC = I // NCORES  # 32 groups per core
NCHUNK = 2
CHUNK = B // NCHUNK  # 512

F16 = mybir.dt.float16
F32 = mybir.dt.float32

N_WARMUP = 12
FILL_PR = 1
FILL_PZ = 1
FILL_PH = 1

_PROGRAM = None


def _build_program():
    nc = bacc.Bacc(
        "TRN2",
        target_bir_lowering=False,
        debug=False,
        enable_asserts=False,
    )

    hT_d = nc.dram_tensor("hT", [GPC, D, B], F16, kind="ExternalInput").ap()
    u_d = nc.dram_tensor("ucat", [D, GPC * 3 * D], F16, kind="ExternalInput").ap()
    # W in row 0, 127 zero rows below (K=128-padded rank-1 stationary).
    wpad_d = nc.dram_tensor("wpad", [D, GPC * 3 * D], F16, kind="ExternalInput").ap()
    xrow_d = nc.dram_tensor("xrow", [GPC, B], F16, kind="ExternalInput").ap()
    # Full [D,B] images (row0=X_g, rest zeros) for the first XBUF groups --
    # loading these via DMA removes the xmov memset from the startup path.
    xfull_d = nc.dram_tensor("xfull", [8, D, B], F16, kind="ExternalInput").ap()
    b_d = nc.dram_tensor("bcat", [D, GPC * 3], F32, kind="ExternalInput").ap()
    hnT_d = nc.dram_tensor("hnT", [GPC, D, B], F16, kind="ExternalOutput").ap()
    htT_d = nc.dram_tensor("htT", [GPC, D, B], F16, kind="ExternalOutput").ap()

    sig = mybir.ActivationFunctionType.Sigmoid
    tanh = mybir.ActivationFunctionType.Tanh

    with tile.TileContext(nc) as tc, ExitStack() as ctx:
        u_pool = ctx.enter_context(tc.tile_pool(name="u", bufs=1))
        wpad_pool = ctx.enter_context(tc.tile_pool(name="wp", bufs=1))
        xmov_pool = ctx.enter_context(tc.tile_pool(name="xm", bufs=1))
        b_pool = ctx.enter_context(tc.tile_pool(name="b", bufs=1))
        warm_pool = ctx.enter_context(tc.tile_pool(name="warm", bufs=1))
        hT_pool = ctx.enter_context(tc.tile_pool(name="hT", bufs=6))
        ps_pool = ctx.enter_context(tc.tile_pool(name="ps", bufs=3, space="PSUM"))
        fill_pool = ctx.enter_context(tc.tile_pool(name="fil", bufs=1, space="PSUM"))
        act_pool = ctx.enter_context(tc.tile_pool(name="act", bufs=4))
        mid_pool = ctx.enter_context(tc.tile_pool(name="mid", bufs=6))
        out_pool = ctx.enter_context(tc.tile_pool(name="out", bufs=4))

        # K=128 zero operands for warmup/filler matmuls.
        zf = warm_pool.tile([D, D + CHUNK], F16)
        nc.vector.memset(zf[:], 0)
        fill_ps = fill_pool.tile([D, CHUNK], F32)

        def filler(n):
            for _ in range(n):
                nc.tensor.matmul(
                    fill_ps[:], lhsT=zf[:, :D], rhs=zf[:, D:],
                    start=True, stop=True, skip_group_check=True,
                )

        filler(N_WARMUP)

        # xmov: 8-buffered [D, B] moving operand for the rank-1 term;
        # rows 1..127 stay zero forever, row 0 gets X_g per group.
        # XBUF must exceed PF+3 so the prefetch for g+PF never lands in a
        # buffer still readable by stageB(g-1).
        XBUF = 8
        xmov = xmov_pool.tile([D, XBUF * B], F16)

        b_sb = b_pool.tile([D, GPC * 3], F32)
        nc.sync.dma_start(b_sb[:], b_d[:])
        u_sb = u_pool.tile([D, GPC * 3 * D], F16)
        wpad_sb = wpad_pool.tile([D, GPC * 3 * D], F16)
        NCHK = 4
        UCH = GPC // NCHK * 3 * D

        def load_chunk(c):
            # u/wpad for one 4-group slice, issued lazily so the first hT
            # DMA isn't queued behind 6 MiB of weights.
            nc.sync.dma_start(u_sb[:, c * UCH : (c + 1) * UCH], u_d[:, c * UCH : (c + 1) * UCH])
            nc.sync.dma_start(
                wpad_sb[:, c * UCH : (c + 1) * UCH], wpad_d[:, c * UCH : (c + 1) * UCH]
            )

        def prefetch(g):
            hT = hT_pool.tile([D, B], F16, tag="hT", name=f"hT{g}")
            nc.sync.dma_start(hT[:], hT_d[g])
            xoff = (g % XBUF) * B
            if g < XBUF:
                nc.sync.dma_start(xmov[:, xoff : xoff + B], xfull_d[g])
            else:
                nc.sync.dma_start(xmov[0:1, xoff : xoff + B], xrow_d[g : g + 1, :])
            return hT

        def gemm(psum, g, gate, moving):
            u_g = u_sb[:, (g * 3 + gate) * D : (g * 3 + gate + 1) * D]
            w_g = wpad_sb[:, (g * 3 + gate) * D : (g * 3 + gate + 1) * D]
            xoff = (g % XBUF) * B
            for c in range(NCHUNK):
                sl = slice(c * CHUNK, (c + 1) * CHUNK)
                nc.tensor.matmul(
                    psum[:, sl], lhsT=u_g, rhs=moving[:, sl],
                    start=True, stop=False,
                )
            for c in range(NCHUNK):
                sl = slice(c * CHUNK, (c + 1) * CHUNK)
                nc.tensor.matmul(
                    psum[:, sl], lhsT=w_g,
                    rhs=xmov[:, xoff + c * CHUNK : xoff + (c + 1) * CHUNK],
                    start=False, stop=True,
                )

        def bias(g, gate):
            return b_sb[:, g * 3 + gate : g * 3 + gate + 1]

        def stageA(g, hT):
            filler(FILL_PR)
            pr = ps_pool.tile([D, B], F32, tag="ps", name=f"pr{g}")
            gemm(pr, g, 0, hT)
            rt = act_pool.tile([D, B], F16, tag="rt", name=f"rt{g}")
            nc.scalar.activation(rt[:], pr[:], sig, bias=bias(g, 0))
            rh = mid_pool.tile([D, B], F16, tag="rh", name=f"rh{g}")
            nc.vector.tensor_mul(rh[:], rt[:], hT[:])

            filler(FILL_PZ)
            pz = ps_pool.tile([D, B], F32, tag="ps", name=f"pz{g}")
            gemm(pz, g, 1, hT)
            zt = act_pool.tile([D, B], F16, tag="zt", name=f"zt{g}")
            nc.scalar.activation(zt[:], pz[:], sig, bias=bias(g, 1))
            return dict(g=g, hT=hT, zt=zt, rh=rh)

        def stageB(s, last=False):
            g = s["g"]
            filler(FILL_PH)
            ht = out_pool.tile([D, B], F16, tag="ht", name=f"ht{g}")
            ph = ps_pool.tile([D, B], F32, tag="ps", name=f"ph{g}")
            gemm(ph, g, 2, s["rh"])
            nc.scalar.activation(ht[:], ph[:], tanh, bias=bias(g, 2))
            nc.gpsimd.dma_start(htT_d[g], ht[:])
            d = mid_pool.tile([D, B], F16, tag="d", name=f"d{g}")
            t = mid_pool.tile([D, B], F16, tag="t", name=f"t{g}")
            hn = out_pool.tile([D, B], F16, tag="hn", name=f"hn{g}")
            # For the last group, process the blend in halves so the final
            # output DMA starts ~1.5us earlier.
            slices = (
                [slice(0, CHUNK), slice(CHUNK, B)] if last else [slice(0, B)]
            )
            for sl in slices:
                nc.vector.tensor_sub(d[:, sl], s["hT"][:, sl], ht[:, sl])
                nc.vector.tensor_mul(t[:, sl], s["zt"][:, sl], d[:, sl])
                nc.vector.tensor_add(hn[:, sl], t[:, sl], ht[:, sl])
                if last:
                    nc.gpsimd.dma_start(hnT_d[g][:, sl], hn[:, sl])
            if not last:
                nc.gpsimd.dma_start(hnT_d[g], hn[:])

        PF = 3  # DMA prefetch distance in groups
        load_chunk(0)
        hTs = {g: prefetch(g) for g in range(PF)}
        pend = None
        for g in range(GPC):
            if g + PF < GPC:
                hTs[g + PF] = prefetch(g + PF)
            if g % 8 == 5 and g // 8 + 1 < NCHK:
                load_chunk(g // 8 + 1)
            cur = stageA(g, hTs.pop(g))
            if pend is not None:
                stageB(pend)
            pend = cur
        stageB(pend, last=True)

    nc.compile()
    return nc


def _get_program():
    global _PROGRAM
    if _PROGRAM is None:
        _PROGRAM = _build_program()
    return _PROGRAM


LAST_EXEC_TIME_NS = None
LAST_RESULTS = None


def kernel(X, h, W_r, W_z, W_h, U_r, U_z, U_h, b_r, b_z, b_h):
    global LAST_EXEC_TIME_NS, LAST_RESULTS
    X = np.asarray(X, dtype=np.float32)
    h = np.asarray(h, dtype=np.float32)
    W = np.stack([np.asarray(W_r), np.asarray(W_z), np.asarray(W_h)], axis=1).astype(
        np.float32
    )  # [I, 3, 1, D]
    U = np.stack([np.asarray(U_r), np.asarray(U_z), np.asarray(U_h)], axis=1).astype(
        np.float32
    )  # [I, 3, D, D]
    bb = np.stack([np.asarray(b_r), np.asarray(b_z), np.asarray(b_h)], axis=1).astype(
        np.float32
    )  # [I, 3, D]

    hT = np.ascontiguousarray(h.reshape(B, I, D).transpose(1, 2, 0)).astype(np.float16)
    XT = np.ascontiguousarray(X.T).astype(np.float16)  # [I, B]

    in_maps = []
    for c in range(NCORES):
        sl = slice(c * GPC, (c + 1) * GPC)
        u_core = U[sl]  # [GPC, 3, D(k), D(d)]
        u_sb = np.ascontiguousarray(
            u_core.transpose(2, 0, 1, 3).reshape(D, GPC * 3 * D)
        ).astype(np.float16)
        wpad = np.zeros((D, GPC * 3 * D), dtype=np.float16)
        wpad[0, :] = W[sl, :, 0, :].reshape(GPC * 3 * D)
        bcat = np.ascontiguousarray(
            bb[sl].reshape(GPC * 3, D).T
        ).astype(np.float32)  # [D, GPC*3]
        xrow = np.ascontiguousarray(XT[sl])
        xfull = np.zeros((8, D, B), dtype=np.float16)
        xfull[:, 0, :] = xrow[:8]
        in_maps.append(
            {
                "hT": np.ascontiguousarray(hT[sl]),
                "ucat": u_sb,
                "wpad": wpad,
                "xrow": xrow,
                "bcat": bcat,
                "xfull": xfull,
            }
        )

    nc = _get_program()
    trace = bool(int(os.environ.get("KERNEL_TRACE", "0")))
    res = run_bass_kernel_spmd(nc, in_maps, core_ids=list(range(NCORES)), trace=trace)
    LAST_EXEC_TIME_NS = res.exec_time_ns
    LAST_RESULTS = res

    hnT = np.concatenate([res.results[c]["hnT"] for c in range(NCORES)], axis=0)
    htT = np.concatenate([res.results[c]["htT"] for c in range(NCORES)], axis=0)
    h_new = (
        np.ascontiguousarray(hnT.transpose(2, 0, 1)).reshape(B, I * D).astype(np.float32)
    )
    h_tilde = (
        np.ascontiguousarray(htT.transpose(2, 0, 1)).reshape(B, I * D).astype(np.float32)
    )
    return h_new, h_tilde
